# revision 16
# baseline (speedup 1.0000x reference)
"""GCN encoder (2x GCNConv + linear projection, relu) on 8 Trainium2 cores.

Self-contained: hardcodes the problem shapes (N=50000, E=800000, C=128,
OUT_C=64) and the sharding strategy.  Host side does structural prep only
(edge partitioning/sorting/padding, index-list construction); all FP math
(matmuls, rsqrt, scaling, aggregation, bias, relu) runs on device.

Math identity used on device, per GCNConv layer:
    g = dinv * (x @ W.T)          (dinv = rsqrt(indeg+1), per node)
    out[d] = relu(dinv[d] * (sum_{e: dst=d} g[src_e] + g[d]) + b)

Device mapping per core (v3):
  - nodes sharded by contiguous range (6250/core, padded to 6272 = 49
    windows of 128)
  - per layer the g table is built COLLECTIVELY: each core computes only
    its own 49 windows of g rows (local matmul + dinv scale); two
    AllGather collectives assemble two "piece" tables in DRAM:
      piece0 = all cores' windows 0..24  (8*3200 = 25600 rows)
      piece1 = all cores' windows 25..48 (8*3072 = 24576 rows)
    Both < 32768 rows so int16 gather indices cover them, and piece0 is
    ready early so its gathers overlap the piece1 collective.
  - edges partitioned by dst owner; per layer processed in two phases
    (piece0 edges, then piece1 edges).  Within a phase, windows are
    processed in groups of 4; ONE dma_gather per (group, piece) emitted
    as PREPARE_ONLY + trigger_dma so the GpSimd engine is never held
    through the transfer (descriptor generation runs even before the
    table collective lands; the trigger carries that dep), and calls
    rotate over the 4 SWDGE queues so their drains proceed concurrently.
  - segment-sum per window: selection-matrix (tensor_scalar is_equal vs
    iota) matmuls accumulating into a [128 dst x 128 feat] fp32 PSUM
    tile; phase-0 result (+ self term g[d]) is flushed to SBUF, phase-1
    re-accumulates and applies the epilogue.
  - PE tail work (transpose + next-layer table row / projection) is
    batched at group end so it does not stall the in-order PE queue
    behind each window's DVE epilogue.
"""

import sys
import numpy as np

for _p in ("/opt/trn_rl_repo",):
    if _p not in sys.path:
        sys.path.append(_p)

import concourse.bacc as bacc
import concourse.tile as tile
from concourse import bass, mybir, bass_utils

F32 = mybir.dt.float32
BF16 = mybir.dt.bfloat16
I16 = mybir.dt.int16
AF = mybir.ActivationFunctionType
ALU = mybir.AluOpType
NP_BF16 = mybir.dt.np(BF16)

N = 50000
E = 800000
C = 128
OUT_C = 64
CORES = 8
S = N // CORES            # 6250 real nodes per shard
NW = 49                   # windows of 128 dst nodes per core
SP = NW * 128             # 6272 padded shard rows
W0 = 25                   # windows in piece 0
W1 = NW - W0              # windows in piece 1
PR0 = W0 * 128            # 3200 piece-0 rows per core
PR1 = W1 * 128            # 3072
T0 = CORES * PR0          # 25600 piece-0 table rows
T1 = CORES * PR1          # 24576
WG = 3                    # windows per gather group
GROUPS = [list(range(s, min(s + WG, NW))) for s in range(0, NW, WG)]
NG = len(GROUPS)


def _wrap16(a):
    """[L] -> [128, L/16] int16 idx layout for dma_gather (16-wrap, 8x repl)."""
    assert a.size % 16 == 0
    w = a.reshape(-1, 16).T.astype(np.int16)
    return np.ascontiguousarray(np.tile(w, (8, 1)))


def _host_prep(x, edge_index):
    """Structural prep: edge partitioning/sorting/padding + index lists."""
    src = np.asarray(edge_index[0]).astype(np.int64)
    dst = np.asarray(edge_index[1]).astype(np.int64)
    deg = np.bincount(dst, minlength=N).astype(np.float32) + 1.0

    owner = dst // S
    loc = dst - owner * S
    win = loc // 128
    rel = (loc % 128).astype(np.float32)
    sc = src // S
    sl = src - sc * S
    pc = (sl >= PR0).astype(np.int64)            # source piece
    prow = np.where(pc == 1, sc * PR1 + sl - PR0, sc * PR0 + sl)

    # common chunk schedule: caps[p][w] = chunks per (window, piece),
    # max over cores so the single SPMD program fits every core
    key = ((owner * NW + win) * 2 + pc)
    counts = np.bincount(key, minlength=CORES * NW * 2).reshape(CORES, NW, 2)
    maxc = counts.max(axis=0)                                  # [NW, 2]
    caps = [[-(-int(maxc[w, p]) // 128) for w in range(NW)] for p in (0, 1)]
    assert all(cc >= 1 for p in (0, 1) for cc in caps[p])
    pad8 = [(-sum(caps[0])) % 8, (-sum(caps[1])) % 8]

    # per-(group, piece) gather call sizes (blocks) and idx column offsets
    gnb = [[sum(caps[p][w] for w in ws) for ws in GROUPS] for p in (0, 1)]
    off16 = {}
    o = 0
    for p in (0, 1):
        for g in range(NG):
            off16[(g, p)] = o
            o += gnb[p][g] * 8            # blocks*128/16 idx cols
    tot16 = o

    degp = np.ones(SP * CORES, np.float32)
    nodes = np.arange(N, dtype=np.int64)
    degp[(nodes // S) * SP + (nodes % S)] = deg
    degt = np.ascontiguousarray(degp.reshape(-1, 128).T)         # [128, NT]
    xpad_all = np.zeros((SP * CORES, C), np.float32)
    xpad_all[(nodes // S) * SP + (nodes % S)] = np.asarray(x, np.float32)
    xt = np.ascontiguousarray(xpad_all.T).astype(NP_BF16)        # [128, NPAD]

    per_core = []
    for c in range(CORES):
        m = owner == c
        cw, cr, cs, cp = win[m], rel[m], prow[m], pc[m]
        order = np.lexsort((cs, cp, cw))
        cw, cr, cs, cp = cw[order], cr[order], cs[order], cp[order]
        k = cw * 2 + cp
        idx_parts = [[], []]
        rel_parts = [[], []]
        for w in range(NW):
            for p in (0, 1):
                lo = np.searchsorted(k, w * 2 + p, "left")
                hi = np.searchsorted(k, w * 2 + p, "right")
                n = hi - lo
                cap = caps[p][w]
                assert n <= cap * 128
                iv = np.concatenate(
                    [cs[lo:hi], np.zeros(cap * 128 - n, np.int64)])
                rv = np.concatenate(
                    [cr[lo:hi], np.full(cap * 128 - n, -1.0, np.float32)])
                idx_parts[p].append(iv)
                rel_parts[p].append(rv)
        # idx stream in call order: (p, group)
        idx_all = []
        for p in (0, 1):
            for ws in GROUPS:
                for w in ws:
                    idx_all.append(idx_parts[p][w])
        idx_all = np.concatenate(idx_all)
        # rel stream in chunk-consumption order: phase 0 windows (+pad to
        # x8 columns for the batched is_equal), then phase 1
        rel_all = np.concatenate(
            rel_parts[0] + [np.full(pad8[0] * 128, -1.0, np.float32)] +
            rel_parts[1] + [np.full(pad8[1] * 128, -1.0, np.float32)])
        relT = np.ascontiguousarray(
            rel_all.reshape(-1, 128).T).astype(NP_BF16)
        degl = np.ascontiguousarray(
            degp[c * SP:(c + 1) * SP].reshape(NW, 128).T)        # [128, NW]
        xpad = np.zeros((SP, C), np.float32)
        xpad[:S] = np.asarray(x, np.float32)[c * S:(c + 1) * S]
        xtl = np.ascontiguousarray(xpad.T).astype(NP_BF16)       # [128, SP]
        per_core.append(dict(idx=_wrap16(idx_all), rel=relT, degl=degl,
                             xtl=xtl))

    sched = dict(caps=caps, gnb=gnb, off16=off16, tot16=tot16, pad8=pad8)
    shared = dict(xt=xt, degt=degt)
    return sched, shared, per_core


def _build_nc(sched):
    caps, gnb, off16, tot16, pad8 = (sched["caps"], sched["gnb"],
                                     sched["off16"], sched["tot16"],
                                     sched["pad8"])
    nb_max = [max(gnb[0]), max(gnb[1])]
    nchcol = sum(caps[0]) + pad8[0] + sum(caps[1]) + pad8[1]

    nc = bacc.Bacc("TRN2", target_bir_lowering=False, debug=False,
                   enable_asserts=False, num_devices=CORES,
                   num_swdge_queues=4)

    def inp(name, shape, dt=F32):
        return nc.dram_tensor(name, shape, dt, kind="ExternalInput").ap()

    xtl_d = inp("xtl", [128, SP], BF16)
    xt_d = inp("xt", [128, CORES * SP], BF16)
    degt_d = inp("degt", [128, CORES * NW])
    w1t_d = inp("w1t", [C, C], BF16)
    w2t_d = inp("w2t", [C, C], BF16)
    wpt_d = inp("wpt", [C, OUT_C], BF16)
    b1b_d = inp("b1b", [128, C])
    b2b_d = inp("b2b", [128, C])
    bpb_d = inp("bpb", [128, OUT_C])
    degl_d = inp("degl", [128, NW])
    iota_d = inp("iota", [128, 8 * 128], BF16)
    ident_d = inp("ident", [128, 128])
    idx_d = inp("idx", [128, tot16], I16)
    rel_d = inp("rel", [128, nchcol], BF16)
    out_d = nc.dram_tensor("out", [SP, OUT_C], F32, kind="ExternalOutput").ap()

    gloc = {}
    gtab = {}
    for p, (pr, tr) in enumerate(((PR0, T0), (PR1, T1))):
        gtab[(1, p)] = nc.dram_tensor(
            f"g1tab{p}", [tr, C], BF16, kind="Internal").ap()
        gloc[(2, p)] = nc.dram_tensor(
            f"g2loc{p}", [pr, C], BF16, kind="Internal").ap()
        gtab[(2, p)] = nc.dram_tensor(
            f"g2tab{p}", [tr, C], BF16, kind="Internal",
            addr_space="Shared").ap()

    from contextlib import ExitStack
    with tile.TileContext(nc) as tc, ExitStack() as ctx:
        cp = ctx.enter_context(tc.tile_pool(name="consts", bufs=1))
        stg0 = ctx.enter_context(tc.tile_pool(name="stg0", bufs=5))
        stg1 = ctx.enter_context(tc.tile_pool(name="stg1", bufs=5))
        spool = ctx.enter_context(tc.tile_pool(name="sel", bufs=6))
        epool = ctx.enter_context(tc.tile_pool(name="epi", bufs=4))
        opool = ctx.enter_context(tc.tile_pool(name="otiles", bufs=1))
        opool2 = ctx.enter_context(tc.tile_pool(name="owin", bufs=6))
        ppw = ctx.enter_context(tc.tile_pool(name="psw", bufs=3, space="PSUM"))
        ppg = ctx.enter_context(tc.tile_pool(name="psg", bufs=2, space="PSUM"))
        ppt = ctx.enter_context(tc.tile_pool(name="pst", bufs=2, space="PSUM"))
        ppp = ctx.enter_context(tc.tile_pool(name="psp", bufs=1, space="PSUM"))

        def cload(name, ap, shape, dt=F32):
            t = cp.tile(shape, dt, tag=name)
            nc.sync.dma_start(t[:], ap[:])
            return t

        # build-critical consts first so L1 gathers can start ASAP
        degl = cload("degl", degl_d, [128, NW])
        degt = cload("degt", degt_d, [128, CORES * NW])
        xtl = cload("xtl", xtl_d, [128, SP], BF16)
        w1t = cload("w1t", w1t_d, [C, C], BF16)
        w2t = cload("w2t", w2t_d, [C, C], BF16)
        sql = cp.tile([128, NW], F32, tag="sql")
        nc.scalar.activation(sql[:], degl[:], AF.Sqrt)
        dinvl = cp.tile([128, NW], F32, tag="dinvl")
        nc.vector.reciprocal(dinvl[:], sql[:])
        sqt = cp.tile([128, CORES * NW], F32, tag="sqt")
        nc.scalar.activation(sqt[:], degt[:], AF.Sqrt)
        dinvt = cp.tile([128, CORES * NW], F32, tag="dinvt")
        nc.vector.reciprocal(dinvt[:], sqt[:])

        # ---- L1 table: every core builds the FULL table from replicated X,
        # piece-0 node tiles first so piece-0 gathers start early.  Loads,
        # scales and stores run in 4-window batches; DMAs issue from the
        # Pool queue (cheapest sequencer cost). ----
        xbp = ctx.enter_context(tc.tile_pool(name="xbuild", bufs=3))
        gbp = ctx.enter_context(tc.tile_pool(name="gbuild", bufs=3))

        def build_piece(p, wlo, whi, pr):
            for cc in range(CORES):
                w = wlo
                while w < whi:
                    nwb = min(4, whi - w)
                    xb = xbp.tile([128, 4 * 128], BF16, tag="xb")
                    c0 = cc * SP + w * 128
                    nc.gpsimd.dma_start(xb[:, :nwb * 128],
                                        xt_d[:, c0:c0 + nwb * 128])
                    gb = gbp.tile([128, 4 * 128], BF16, tag="gbb")
                    for j in range(nwb):
                        t = cc * NW + w + j
                        ps = ppg.tile([128, C], F32, tag="psg")
                        nc.tensor.matmul(ps[:],
                                         lhsT=xb[:, j * 128:(j + 1) * 128],
                                         rhs=w1t[:], start=True, stop=True)
                        dsl = dinvt[:, t:t + 1]
                        if t % 2 == 0:
                            nc.vector.tensor_tensor(
                                out=gb[:, j * 128:(j + 1) * 128], in0=ps[:],
                                in1=dsl.to_broadcast([128, 128]), op=ALU.mult)
                        else:
                            nc.scalar.activation(
                                gb[:, j * 128:(j + 1) * 128], ps[:],
                                AF.Identity, scale=dsl)
                    r0 = cc * pr + (w - wlo) * 128
                    nc.gpsimd.dma_start(
                        gtab[(1, p)][r0:r0 + nwb * 128, :].rearrange(
                            "(j p) f -> p j f", p=128),
                        gb[:, :nwb * 128].rearrange("p (j f) -> p j f", f=C))
                    w += nwb

        build_piece(0, 0, W0, PR0)

        # self-term tiles from the core's own shard
        gl = {}
        for w in range(NW):
            ps = ppg.tile([128, C], F32, tag="psg")
            nc.tensor.matmul(ps[:], lhsT=xtl[:, w * 128:(w + 1) * 128],
                             rhs=w1t[:], start=True, stop=True)
            glw = opool.tile([128, C], F32, tag=f"gl_{w}")
            nc.scalar.activation(glw[:], ps[:], AF.Identity,
                                 scale=dinvl[:, w:w + 1])
            gl[w] = glw

        build_piece(1, W0, NW, PR1)

        # remaining consts (overlap the first collective)
        wpt = cload("wpt", wpt_d, [C, OUT_C], BF16)
        b1b = cload("b1b", b1b_d, [128, C])
        b2b = cload("b2b", b2b_d, [128, C])
        bpb = cload("bpb", bpb_d, [128, OUT_C])
        iota = cload("iota", iota_d, [128, 8 * 128], BF16)
        ident = cload("ident", ident_d, [128, 128])
        idx = cload("idx", idx_d, [128, tot16], I16)
        rel = cload("rel", rel_d, [128, nchcol], BF16)

        state = dict(ci=0, qi=0)
        parts = {}

        selb_cur = [None]

        def sel_for(ci):
            if ci % 8 == 0:
                sb = spool.tile([128, 8 * 128], BF16, tag="selb")
                nc.vector.tensor_tensor(
                    out=sb[:].rearrange("p (c f) -> p c f", f=128),
                    in0=rel[:, ci:ci + 8].rearrange(
                        "p (c o) -> p c o", o=1).to_broadcast([128, 8, 128]),
                    in1=iota[:].rearrange("p (c f) -> p c f", f=128),
                    op=ALU.is_equal)
                selb_cur[0] = sb
            k = ci % 8
            return selb_cur[0][:, k * 128:(k + 1) * 128]

        def phase(L, p, post_window, tail=None):
            """One gather+aggregate phase: piece p edges of layer L."""
            if p == 0:
                state["ci"] = 0          # rel stream is shared by both layers
            pool = stg0 if p == 0 else stg1
            for g, ws in enumerate(GROUPS):
                nb = gnb[p][g]
                stg = pool.tile([128, nb_max[p], C], BF16, tag="stg")
                nidx = nb * 128
                o16 = off16[(g, p)]
                q = state["qi"] % 4
                state["qi"] += 1
                nc.gpsimd.dma_gather(
                    stg[:, :nb, :], gtab[(L, p)],
                    idx[:, o16:o16 + nidx // 16], nidx, nidx, elem_size=C,
                    single_packet=False, queue_num=q)
                blk = 0
                os = []
                for w in ws:
                    cap = caps[p][w]
                    ps = ppw.tile([128, C], F32, tag="psw")
                    for j in range(cap):
                        sel = sel_for(state["ci"])
                        state["ci"] += 1
                        nc.tensor.matmul(ps[:], lhsT=sel, rhs=stg[:, blk, :],
                                         start=(j == 0), stop=(j == cap - 1))
                        blk += 1
                    os.append((w, post_window(w, ps)))
                if tail is not None:
                    for w, o in os:
                        tail(w, o)
            state["ci"] += pad8[p]

        def flush_partial(w, ps):
            """Phase-0 epilogue: partial = psum + self-term g[d]."""
            part = opool.tile([128, C], F32, tag=f"part_{w}")
            nc.vector.tensor_tensor(out=part[:], in0=ps[:], in1=gl[w][:],
                                    op=ALU.add)
            parts[w] = part

        def final_epilogue(w, ps, bias_sb):
            """Phase-1 epilogue: relu(dinv*(ps + partial) + bias)."""
            t1 = epool.tile([128, C], F32, tag="t1")
            nc.vector.tensor_tensor(out=t1[:], in0=ps[:], in1=parts[w][:],
                                    op=ALU.add)
            t2 = epool.tile([128, C], F32, tag="t2")
            nc.scalar.activation(t2[:], t1[:], AF.Identity,
                                 scale=dinvl[:, w:w + 1])
            t3 = epool.tile([128, C], F32, tag="t3")
            nc.vector.tensor_tensor(out=t3[:], in0=t2[:], in1=bias_sb[:],
                                    op=ALU.add)
            o = opool2.tile([128, C], F32, tag="otile")
            nc.scalar.activation(o[:], t3[:], AF.Relu)
            return o

        def l1_tail(w, o):
            """Transpose h1 and build the L2 table row for window w."""
            pst = ppt.tile([128, 128], F32, tag="pst")
            nc.tensor.transpose(pst[:], o[:], ident[:])
            h1t = epool.tile([128, 128], BF16, tag="h1t")
            nc.vector.tensor_copy(h1t[:], pst[:])
            ps2 = ppg.tile([128, C], F32, tag="psg")
            nc.tensor.matmul(ps2[:], lhsT=h1t[:], rhs=w2t[:],
                             start=True, stop=True)
            gb = epool.tile([128, C], BF16, tag="gb")
            nc.scalar.activation(gb[:], ps2[:], AF.Identity,
                                 scale=dinvl[:, w:w + 1])
            p = int(w >= W0)
            r0 = (w - (W0 if p else 0)) * 128
            nc.sync.dma_start(gloc[(2, p)][r0:r0 + 128, :], gb[:])
            glw = opool.tile([128, C], F32, tag=f"gl_{w}")
            nc.scalar.activation(glw[:], ps2[:], AF.Identity,
                                 scale=dinvl[:, w:w + 1])
            gl[w] = glw
            if w == W0 - 1:
                nc.gpsimd.collective_compute(
                    "AllGather", ALU.bypass,
                    replica_groups=[list(range(CORES))],
                    ins=[gloc[(2, 0)][:]], outs=[gtab[(2, 0)][:]])
            if w == NW - 1:
                nc.gpsimd.collective_compute(
                    "AllGather", ALU.bypass,
                    replica_groups=[list(range(CORES))],
                    ins=[gloc[(2, 1)][:]], outs=[gtab[(2, 1)][:]])

        def l2_tail(w, o):
            """Project and store output rows for window w."""
            pst = ppt.tile([128, 128], F32, tag="pst")
            nc.tensor.transpose(pst[:], o[:], ident[:])
            h2t = epool.tile([128, 128], BF16, tag="h2t")
            nc.vector.tensor_copy(h2t[:], pst[:])
            psp = ppp.tile([128, OUT_C], F32, tag="psp")
            nc.tensor.matmul(psp[:], lhsT=h2t[:], rhs=wpt[:],
                             start=True, stop=True)
            of = epool.tile([128, OUT_C], F32, tag="of")
            nc.vector.tensor_tensor(out=of[:], in0=psp[:], in1=bpb[:],
                                    op=ALU.add)
            ofr = epool.tile([128, OUT_C], F32, tag="ofr")
            nc.scalar.activation(ofr[:], of[:], AF.Relu)
            nc.sync.dma_start(out_d[w * 128:(w + 1) * 128, :], ofr[:])

        phase(1, 0, flush_partial)
        phase(1, 1, lambda w, ps: final_epilogue(w, ps, b1b), l1_tail)
        phase(2, 0, flush_partial)
        phase(2, 1, lambda w, ps: final_epilogue(w, ps, b2b), l2_tail)

    nc.compile()
    return nc


def _make_in_maps(sched, shared, per_core, W1, b1, W2, b2, Wp, bp):
    w1t = np.ascontiguousarray(np.asarray(W1, np.float32).T).astype(NP_BF16)
    w2t = np.ascontiguousarray(np.asarray(W2, np.float32).T).astype(NP_BF16)
    wpt = np.ascontiguousarray(np.asarray(Wp, np.float32).T).astype(NP_BF16)
    b1b = np.ascontiguousarray(np.tile(np.asarray(b1, np.float32)[None], (128, 1)))
    b2b = np.ascontiguousarray(np.tile(np.asarray(b2, np.float32)[None], (128, 1)))
    bpb = np.ascontiguousarray(np.tile(np.asarray(bp, np.float32)[None], (128, 1)))
    iota = np.ascontiguousarray(
        np.tile(np.arange(128, dtype=np.float32)[None, :],
                (128, 8))).astype(NP_BF16)
    ident = np.eye(128, dtype=np.float32)
    base = dict(w1t=w1t, w2t=w2t, wpt=wpt, b1b=b1b, b2b=b2b, bpb=bpb,
                iota=iota, ident=ident, xt=shared["xt"], degt=shared["degt"])
    in_maps = []
    for c in range(CORES):
        pc = per_core[c]
        m = dict(base)
        m["idx"] = pc["idx"]
        m["rel"] = pc["rel"]
        m["degl"] = pc["degl"]
        m["xtl"] = pc["xtl"]
        in_maps.append(m)
    return in_maps


def _run(inputs, trace=False, tmpdir=None, verbose=True):
    import time
    t0 = time.time()
    def _log(msg):
        if verbose:
            print(f"[kernel {time.time()-t0:7.1f}s] {msg}", flush=True)
    sched, shared, per_core = _host_prep(inputs["x"], inputs["edge_index"])
    _log("host prep done")
    nc = _build_nc(sched)
    _log("build+compile done")
    in_maps = _make_in_maps(sched, shared, per_core,
                            inputs["W1"], inputs["b1"], inputs["W2"],
                            inputs["b2"], inputs["Wp"], inputs["bp"])
    _log("in_maps done")
    core_ids = list(range(CORES))
    if trace:
        # NTFF profiling needs a warm first execute; run once untraced.
        bass_utils.run_bass_kernel_spmd(nc, in_maps, core_ids=core_ids,
                                        trace=False)
        _log("warmup run done")
    res = bass_utils.run_bass_kernel_spmd(nc, in_maps, core_ids=core_ids,
                                          trace=trace, tmpdir=tmpdir)
    _log("run done")
    out = np.empty((N, OUT_C), np.float32)
    for c in range(CORES):
        out[c * S:(c + 1) * S] = res.results[c]["out"][:S]
    return out, res


def kernel(**inputs):
    out, _ = _run(inputs)
    return out


# revision 18
# speedup vs baseline: 1.2402x; 1.2402x over previous
"""GCN encoder (2x GCNConv + linear projection, relu) on 8 Trainium2 cores.

Self-contained: hardcodes the problem shapes (N=50000, E=800000, C=128,
OUT_C=64) and the sharding strategy.  Host side does structural prep only
(edge partitioning/sorting/padding, index-list construction); all FP math
(matmuls, rsqrt, scaling, aggregation, bias, relu) runs on device.

Math identity used on device, per GCNConv layer:
    g = dinv * (x @ W.T)          (dinv = rsqrt(indeg+1), per node)
    out[d] = relu(dinv[d] * (sum_{e: dst=d} g[src_e] + g[d]) + b)
           = relu(dinv[d] * (agg + g[d] + b*sqrt(deg[d])))

Device mapping per core (v8):
  - nodes sharded by contiguous range (6250/core, padded to 6272 = 49
    windows of 128); gather tables split in two pieces so int16 indices
    cover them:
      piece0 = all cores' windows 0..24  (8*3200 = 25600 rows)
      piece1 = all cores' windows 25..48 (8*3072 = 24576 rows)
  - L1 tables: every core builds the FULL table from the replicated X
    (piece-0 node tiles first; the piece-1 build is interleaved into the
    piece-0 gather phase as bubble-filler, with its DMAs on the SP queue
    so the Pool queue stays clear for gathers).
  - L2 tables: AllGather collectives of locally computed rows, fired as
    soon as windows 0..24 / 25..48 finish layer 1.
  - edges partitioned by dst owner, grouped in 4-window gather groups;
    ONE dma_gather per (group, piece) rotating over the 4 SWDGE queues.
    Layer 1 runs two phases (piece0 edges while the piece1 table builds,
    then piece1 edges); layer 2 runs a single phase per window (both
    pieces into one PSUM accumulation).
  - segment-sum per window: selection-matrix matmuls (is_equal vs iota,
    built 8 chunks per DVE op) accumulating into a [128 dst x 128 feat]
    fp32 PSUM tile.
  - epilogue: relu(dinv * (ps + carry + bias*sqrt(deg))) with the
    bias*sqrt tiles precomputed per window, so each window costs two DVE
    adds + one fused ACT relu-scale.
"""

import sys
import numpy as np

for _p in ("/opt/trn_rl_repo",):
    if _p not in sys.path:
        sys.path.append(_p)

import concourse.bacc as bacc
import concourse.tile as tile
from concourse import bass, mybir, bass_utils

F32 = mybir.dt.float32
BF16 = mybir.dt.bfloat16
I16 = mybir.dt.int16
AF = mybir.ActivationFunctionType
ALU = mybir.AluOpType
NP_BF16 = mybir.dt.np(BF16)

N = 50000
E = 800000
C = 128
OUT_C = 64
CORES = 8
S = N // CORES            # 6250 real nodes per shard
NW = 49                   # windows of 128 dst nodes per core
SP = NW * 128             # 6272 padded shard rows
NT = CORES * NW           # node tiles in the full padded table
W0 = 25                   # windows in piece 0
W1 = NW - W0
PR0 = W0 * 128
PR1 = W1 * 128
T0 = CORES * PR0          # 25600 piece-0 table rows
T1 = CORES * PR1          # 24576
WG = 4                    # windows per gather group
GROUPS = [list(range(s, min(s + WG, NW))) for s in range(0, NW, WG)]
NG = len(GROUPS)          # 13


def _wrap16(a):
    """[L] -> [128, L/16] int16 idx layout for dma_gather (16-wrap, 8x repl)."""
    assert a.size % 16 == 0
    w = a.reshape(-1, 16).T.astype(np.int16)
    return np.ascontiguousarray(np.tile(w, (8, 1)))


def _host_prep(x, edge_index):
    """Structural prep: edge partitioning/sorting/padding + index lists."""
    src = np.asarray(edge_index[0]).astype(np.int64)
    dst = np.asarray(edge_index[1]).astype(np.int64)
    deg = np.bincount(dst, minlength=N).astype(np.float32) + 1.0

    owner = dst // S
    loc = dst - owner * S
    win = loc // 128
    rel = (loc % 128).astype(np.float32)
    sc = src // S
    sl = src - sc * S
    pc = (sl >= PR0).astype(np.int64)            # source piece
    prow = np.where(pc == 1, sc * PR1 + sl - PR0, sc * PR0 + sl)

    # common chunk schedule: caps[p][w] = chunks per (window, piece),
    # max over cores so the single SPMD program fits every core
    key = ((owner * NW + win) * 2 + pc)
    counts = np.bincount(key, minlength=CORES * NW * 2).reshape(CORES, NW, 2)
    maxc = counts.max(axis=0)                                  # [NW, 2]
    caps = [[-(-int(maxc[w, p]) // 128) for w in range(NW)] for p in (0, 1)]
    assert all(cc >= 1 for p in (0, 1) for cc in caps[p])

    # per-(group, piece) gather call sizes (blocks) and idx column offsets
    gnb = [[sum(caps[p][w] for w in ws) for ws in GROUPS] for p in (0, 1)]
    off16 = {}
    o = 0
    for p in (0, 1):
        for g in range(NG):
            off16[(g, p)] = o
            o += gnb[p][g] * 8            # blocks*128/16 idx cols
    tot16 = o

    # rel streams: A = phase-ordered (L1: all piece0 chunks then piece1,
    # each padded to x8 columns); B = window-ordered (L2: per window
    # piece0 chunks then piece1 chunks, padded to x8 at the end)
    nch = [sum(caps[0]), sum(caps[1])]
    pad8 = [(-nch[0]) % 8, (-nch[1]) % 8]
    lenA = nch[0] + pad8[0] + nch[1] + pad8[1]
    nchB = nch[0] + nch[1]
    padB = (-nchB) % 8
    nchcol = lenA + nchB + padB

    degp = np.ones(SP * CORES, np.float32)
    nodes = np.arange(N, dtype=np.int64)
    degp[(nodes // S) * SP + (nodes % S)] = deg
    degt = np.ascontiguousarray(degp.reshape(-1, 128).T)         # [128, NT]
    xpad_all = np.zeros((SP * CORES, C), np.float32)
    xpad_all[(nodes // S) * SP + (nodes % S)] = np.asarray(x, np.float32)
    xt = np.ascontiguousarray(xpad_all.T).astype(NP_BF16)        # [128, NPAD]

    per_core = []
    for c in range(CORES):
        m = owner == c
        cw, cr, cs, cp = win[m], rel[m], prow[m], pc[m]
        order = np.lexsort((cs, cp, cw))
        cw, cr, cs, cp = cw[order], cr[order], cs[order], cp[order]
        k = cw * 2 + cp
        idx_parts = [[], []]
        rel_parts = [[], []]
        for w in range(NW):
            for p in (0, 1):
                lo = np.searchsorted(k, w * 2 + p, "left")
                hi = np.searchsorted(k, w * 2 + p, "right")
                n = hi - lo
                cap = caps[p][w]
                assert n <= cap * 128
                iv = np.concatenate(
                    [cs[lo:hi], np.zeros(cap * 128 - n, np.int64)])
                rv = np.concatenate(
                    [cr[lo:hi], np.full(cap * 128 - n, -1.0, np.float32)])
                idx_parts[p].append(iv)
                rel_parts[p].append(rv)
        idx_all = []
        for p in (0, 1):
            for ws in GROUPS:
                for w in ws:
                    idx_all.append(idx_parts[p][w])
        idx_all = np.concatenate(idx_all)
        relA = np.concatenate(
            rel_parts[0] + [np.full(pad8[0] * 128, -1.0, np.float32)] +
            rel_parts[1] + [np.full(pad8[1] * 128, -1.0, np.float32)])
        relB = np.concatenate(
            [np.concatenate([rel_parts[0][w], rel_parts[1][w]])
             for w in range(NW)] +
            [np.full(padB * 128, -1.0, np.float32)])
        rel_all = np.concatenate([relA, relB])
        relT = np.ascontiguousarray(
            rel_all.reshape(-1, 128).T).astype(NP_BF16)
        degl = np.ascontiguousarray(
            degp[c * SP:(c + 1) * SP].reshape(NW, 128).T)        # [128, NW]
        xpad = np.zeros((SP, C), np.float32)
        xpad[:S] = np.asarray(x, np.float32)[c * S:(c + 1) * S]
        xtl = np.ascontiguousarray(xpad.T).astype(NP_BF16)       # [128, SP]
        per_core.append(dict(idx=_wrap16(idx_all), rel=relT, degl=degl,
                             xtl=xtl))

    sched = dict(caps=caps, gnb=gnb, off16=off16, tot16=tot16, pad8=pad8,
                 lenA=lenA, nchcol=nchcol)
    shared = dict(xt=xt, degt=degt)
    return sched, shared, per_core


def _build_nc(sched):
    caps, gnb, off16, tot16 = (sched["caps"], sched["gnb"],
                               sched["off16"], sched["tot16"])
    pad8, lenA, nchcol = sched["pad8"], sched["lenA"], sched["nchcol"]
    nb_max = [max(gnb[0]), max(gnb[1])]

    nc = bacc.Bacc("TRN2", target_bir_lowering=False, debug=False,
                   enable_asserts=False, num_devices=CORES,
                   num_swdge_queues=4)

    def inp(name, shape, dt=F32):
        return nc.dram_tensor(name, shape, dt, kind="ExternalInput").ap()

    xtl_d = inp("xtl", [128, SP], BF16)
    xt_d = inp("xt", [128, CORES * SP], BF16)
    degt_d = inp("degt", [128, NT])
    w1t_d = inp("w1t", [C, C], BF16)
    w2t_d = inp("w2t", [C, C], BF16)
    wpt_d = inp("wpt", [C, OUT_C], BF16)
    b1b_d = inp("b1b", [128, C])
    b2b_d = inp("b2b", [128, C])
    bpb_d = inp("bpb", [128, OUT_C])
    degl_d = inp("degl", [128, NW])
    iota_d = inp("iota", [128, 8 * 128], BF16)
    ident_d = inp("ident", [128, 128])
    idx_d = inp("idx", [128, tot16], I16)
    rel_d = inp("rel", [128, nchcol], BF16)
    out_d = nc.dram_tensor("out", [SP, OUT_C], F32, kind="ExternalOutput").ap()

    gtab = {}
    gloc = {}
    for p, (pr, tr) in enumerate(((PR0, T0), (PR1, T1))):
        gtab[(1, p)] = nc.dram_tensor(
            f"g1tab{p}", [tr, C], BF16, kind="Internal").ap()
        gloc[(2, p)] = nc.dram_tensor(
            f"g2loc{p}", [pr, C], BF16, kind="Internal").ap()
        gtab[(2, p)] = nc.dram_tensor(
            f"g2tab{p}", [tr, C], BF16, kind="Internal",
            addr_space="Shared").ap()

    from contextlib import ExitStack
    with tile.TileContext(nc) as tc, ExitStack() as ctx:
        cp = ctx.enter_context(tc.tile_pool(name="consts", bufs=1))
        stg0 = ctx.enter_context(tc.tile_pool(name="stg0", bufs=4))
        stg1 = ctx.enter_context(tc.tile_pool(name="stg1", bufs=4))
        xbp = ctx.enter_context(tc.tile_pool(name="xbuild", bufs=3))
        gbp = ctx.enter_context(tc.tile_pool(name="gbuild", bufs=3))
        spool = ctx.enter_context(tc.tile_pool(name="sel", bufs=6))
        epool = ctx.enter_context(tc.tile_pool(name="epi", bufs=4))
        opool = ctx.enter_context(tc.tile_pool(name="otiles", bufs=1))
        opool2 = ctx.enter_context(tc.tile_pool(name="owin", bufs=6))
        ppw = ctx.enter_context(tc.tile_pool(name="psw", bufs=3, space="PSUM"))
        ppg = ctx.enter_context(tc.tile_pool(name="psg", bufs=2, space="PSUM"))
        ppt = ctx.enter_context(tc.tile_pool(name="pst", bufs=2, space="PSUM"))
        ppp = ctx.enter_context(tc.tile_pool(name="psp", bufs=1, space="PSUM"))

        def cload(name, ap, shape, dt=F32):
            t = cp.tile(shape, dt, tag=name)
            nc.sync.dma_start(t[:], ap[:])
            return t

        # build-critical consts first so L1 piece-0 gathers start ASAP
        degl = cload("degl", degl_d, [128, NW])
        degt = cload("degt", degt_d, [128, NT])
        xtl = cload("xtl", xtl_d, [128, SP], BF16)
        w1t = cload("w1t", w1t_d, [C, C], BF16)
        w2t = cload("w2t", w2t_d, [C, C], BF16)
        b1b = cload("b1b", b1b_d, [128, C])
        b2b = cload("b2b", b2b_d, [128, C])
        idx = cload("idx", idx_d, [128, tot16], I16)
        rel = cload("rel", rel_d, [128, nchcol], BF16)
        iota = cload("iota", iota_d, [128, 8 * 128], BF16)

        sql = cp.tile([128, NW], F32, tag="sql")
        nc.scalar.activation(sql[:], degl[:], AF.Sqrt)
        dinvl = cp.tile([128, NW], F32, tag="dinvl")
        nc.vector.reciprocal(dinvl[:], sql[:])
        sqt = cp.tile([128, NT], F32, tag="sqt")
        nc.scalar.activation(sqt[:], degt[:], AF.Sqrt)
        dinvt = cp.tile([128, NT], F32, tag="dinvt")
        nc.vector.reciprocal(dinvt[:], sqt[:])

        # remaining consts
        wpt = cload("wpt", wpt_d, [C, OUT_C], BF16)
        bpb = cload("bpb", bpb_d, [128, OUT_C])
        ident = cload("ident", ident_d, [128, 128])

        # ---- L1 full-table build (from replicated X) ----
        def build_batches(p):
            """Thunks, each building a <=4-window batch of table rows."""
            wlo, whi, pr = (0, W0, PR0) if p == 0 else (W0, NW, PR1)
            out = []
            for cc in range(CORES):
                w = wlo
                while w < whi:
                    nwb = min(4, whi - w)

                    def work(cc=cc, w=w, nwb=nwb):
                        xb = xbp.tile([128, 4 * 128], BF16, tag="xb")
                        c0 = cc * SP + w * 128
                        nc.sync.dma_start(xb[:, :nwb * 128],
                                          xt_d[:, c0:c0 + nwb * 128])
                        gb = gbp.tile([128, 4 * 128], BF16, tag="gbb")
                        for j in range(nwb):
                            t = cc * NW + w + j
                            ps = ppg.tile([128, C], F32, tag="psg")
                            nc.tensor.matmul(
                                ps[:], lhsT=xb[:, j * 128:(j + 1) * 128],
                                rhs=w1t[:], start=True, stop=True)
                            dsl = dinvt[:, t:t + 1]
                            if t % 2 == 0:
                                nc.vector.tensor_tensor(
                                    out=gb[:, j * 128:(j + 1) * 128],
                                    in0=ps[:],
                                    in1=dsl.to_broadcast([128, 128]),
                                    op=ALU.mult)
                            else:
                                nc.scalar.activation(
                                    gb[:, j * 128:(j + 1) * 128], ps[:],
                                    AF.Identity, scale=dsl)
                        r0 = cc * pr + (w - wlo) * 128
                        nc.sync.dma_start(
                            gtab[(1, p)][r0:r0 + nwb * 128, :].rearrange(
                                "(j p) f -> p j f", p=128),
                            gb[:, :nwb * 128].rearrange(
                                "p (j f) -> p j f", f=C))
                    out.append(work)
                    w += nwb
            return out

        for work in build_batches(0):
            work()

        # bias*sqrt(deg) tiles (per window; recomputed per layer) and the
        # self-term tiles for layer 1
        gl = {}
        bw = {}
        for w in range(NW):
            ps = ppg.tile([128, C], F32, tag="psg")
            nc.tensor.matmul(ps[:], lhsT=xtl[:, w * 128:(w + 1) * 128],
                             rhs=w1t[:], start=True, stop=True)
            glw = opool.tile([128, C], F32, tag=f"gl_{w}")
            nc.scalar.activation(glw[:], ps[:], AF.Identity,
                                 scale=dinvl[:, w:w + 1])
            gl[w] = glw
            bww = opool.tile([128, C], BF16, tag=f"bw_{w}")
            nc.scalar.activation(bww[:], b1b[:], AF.Identity,
                                 scale=sql[:, w:w + 1])
            bw[w] = bww

        p1_filler = build_batches(1)

        state = dict(ci=0, qi=0)
        parts = {}
        selb_cur = [None]

        def sel_for(ci):
            if ci % 8 == 0:
                sb = spool.tile([128, 8 * 128], BF16, tag="selb")
                nc.vector.tensor_tensor(
                    out=sb[:].rearrange("p (c f) -> p c f", f=128),
                    in0=rel[:, ci:ci + 8].rearrange(
                        "p (c o) -> p c o", o=1).to_broadcast([128, 8, 128]),
                    in1=iota[:].rearrange("p (c f) -> p c f", f=128),
                    op=ALU.is_equal)
                selb_cur[0] = sb
            k = ci % 8
            return selb_cur[0][:, k * 128:(k + 1) * 128]

        def gather_call(L, p, g):
            nb = gnb[p][g]
            pool = stg0 if p == 0 else stg1
            stg = pool.tile([128, nb_max[p], C], BF16, tag="stg")
            nidx = nb * 128
            o16 = off16[(g, p)]
            q = state["qi"] % 4
            state["qi"] += 1
            nc.gpsimd.dma_gather(
                stg[:, :nb, :], gtab[(L, p)], idx[:, o16:o16 + nidx // 16],
                nidx, nidx, elem_size=C, single_packet=False, queue_num=q)
            return stg

        def agg_window(ps, stg, blk, cap, first, last):
            for j in range(cap):
                sel = sel_for(state["ci"])
                state["ci"] += 1
                nc.tensor.matmul(ps[:], lhsT=sel, rhs=stg[:, blk + j, :],
                                 start=(first and j == 0),
                                 stop=(last and j == cap - 1))
            return blk + cap

        def flush_partial(w, ps):
            part = opool.tile([128, C], F32, tag=f"part_{w}")
            nc.vector.tensor_tensor(out=part[:], in0=ps[:], in1=gl[w][:],
                                    op=ALU.add)
            parts[w] = part

        def final_epilogue(w, ps, carry):
            """relu(dinv * (ps + carry + bias*sqrt(deg)))."""
            t1 = epool.tile([128, C], F32, tag="t1")
            nc.vector.tensor_tensor(out=t1[:], in0=ps[:], in1=carry[:],
                                    op=ALU.add)
            t2 = epool.tile([128, C], F32, tag="t2")
            nc.vector.tensor_tensor(out=t2[:], in0=t1[:], in1=bw[w][:],
                                    op=ALU.add)
            o = opool2.tile([128, C], F32, tag="otile")
            nc.scalar.activation(o[:], t2[:], AF.Relu,
                                 scale=dinvl[:, w:w + 1])
            return o

        def l1_tail(w, o):
            """Transpose h1, build the L2 table row + L2 bias/self tiles."""
            pst = ppt.tile([128, 128], F32, tag="pst")
            nc.tensor.transpose(pst[:], o[:], ident[:])
            h1t = epool.tile([128, 128], BF16, tag="h1t")
            nc.vector.tensor_copy(h1t[:], pst[:])
            ps2 = ppg.tile([128, C], F32, tag="psg")
            nc.tensor.matmul(ps2[:], lhsT=h1t[:], rhs=w2t[:],
                             start=True, stop=True)
            gb = epool.tile([128, C], BF16, tag="gb")
            nc.scalar.activation(gb[:], ps2[:], AF.Identity,
                                 scale=dinvl[:, w:w + 1])
            p = int(w >= W0)
            r0 = (w - (W0 if p else 0)) * 128
            nc.sync.dma_start(gloc[(2, p)][r0:r0 + 128, :], gb[:])
            glw = opool.tile([128, C], F32, tag=f"gl_{w}")
            nc.scalar.activation(glw[:], ps2[:], AF.Identity,
                                 scale=dinvl[:, w:w + 1])
            gl[w] = glw
            bww = opool.tile([128, C], BF16, tag=f"bw_{w}")
            nc.scalar.activation(bww[:], b2b[:], AF.Identity,
                                 scale=sql[:, w:w + 1])
            bw[w] = bww
            if w == W0 - 1:
                nc.gpsimd.collective_compute(
                    "AllGather", ALU.bypass,
                    replica_groups=[list(range(CORES))],
                    ins=[gloc[(2, 0)][:]], outs=[gtab[(2, 0)][:]])
            if w == NW - 1:
                nc.gpsimd.collective_compute(
                    "AllGather", ALU.bypass,
                    replica_groups=[list(range(CORES))],
                    ins=[gloc[(2, 1)][:]], outs=[gtab[(2, 1)][:]])

        def l2_tail(w, o):
            """Project and store output rows for window w."""
            pst = ppt.tile([128, 128], F32, tag="pst")
            nc.tensor.transpose(pst[:], o[:], ident[:])
            h2t = epool.tile([128, 128], BF16, tag="h2t")
            nc.vector.tensor_copy(h2t[:], pst[:])
            psp = ppp.tile([128, OUT_C], F32, tag="psp")
            nc.tensor.matmul(psp[:], lhsT=h2t[:], rhs=wpt[:],
                             start=True, stop=True)
            of = epool.tile([128, OUT_C], F32, tag="of")
            nc.vector.tensor_tensor(out=of[:], in0=psp[:], in1=bpb[:],
                                    op=ALU.add)
            ofr = epool.tile([128, OUT_C], F32, tag="ofr")
            nc.scalar.activation(ofr[:], of[:], AF.Relu)
            nc.sync.dma_start(out_d[w * 128:(w + 1) * 128, :], ofr[:])

        # ---- L1 phase 0 (piece-0 edges), piece-1 build as bubble filler --
        fi = 0
        for g, ws in enumerate(GROUPS):
            stg = gather_call(1, 0, g)
            blk = 0
            for w in ws:
                ps = ppw.tile([128, C], F32, tag="psw")
                blk = agg_window(ps, stg, blk, caps[0][w], True, True)
                flush_partial(w, ps)
            for _ in range(3):
                if fi < len(p1_filler):
                    p1_filler[fi]()
                    fi += 1
        while fi < len(p1_filler):
            p1_filler[fi]()
            fi += 1
        state["ci"] += pad8[0]

        # ---- L1 phase 1 (piece-1 edges) ----
        for g, ws in enumerate(GROUPS):
            stg = gather_call(1, 1, g)
            blk = 0
            os = []
            for w in ws:
                ps = ppw.tile([128, C], F32, tag="psw")
                blk = agg_window(ps, stg, blk, caps[1][w], True, True)
                os.append((w, final_epilogue(w, ps, parts[w])))
            for w, o in os:
                l1_tail(w, o)
        state["ci"] += pad8[1]
        assert state["ci"] == lenA

        # ---- L2: single phase, both pieces per window ----
        for g, ws in enumerate(GROUPS):
            s0 = gather_call(2, 0, g)
            s1 = gather_call(2, 1, g)
            b0 = b1 = 0
            os = []
            for w in ws:
                ps = ppw.tile([128, C], F32, tag="psw")
                b0 = agg_window(ps, s0, b0, caps[0][w], True, False)
                b1 = agg_window(ps, s1, b1, caps[1][w], False, True)
                os.append((w, final_epilogue(w, ps, gl[w])))
            for w, o in os:
                l2_tail(w, o)

    nc.compile()
    return nc


def _make_in_maps(sched, shared, per_core, W1, b1, W2, b2, Wp, bp):
    w1t = np.ascontiguousarray(np.asarray(W1, np.float32).T).astype(NP_BF16)
    w2t = np.ascontiguousarray(np.asarray(W2, np.float32).T).astype(NP_BF16)
    wpt = np.ascontiguousarray(np.asarray(Wp, np.float32).T).astype(NP_BF16)
    b1b = np.ascontiguousarray(np.tile(np.asarray(b1, np.float32)[None], (128, 1)))
    b2b = np.ascontiguousarray(np.tile(np.asarray(b2, np.float32)[None], (128, 1)))
    bpb = np.ascontiguousarray(np.tile(np.asarray(bp, np.float32)[None], (128, 1)))
    iota = np.ascontiguousarray(
        np.tile(np.arange(128, dtype=np.float32)[None, :],
                (128, 8))).astype(NP_BF16)
    ident = np.eye(128, dtype=np.float32)
    base = dict(w1t=w1t, w2t=w2t, wpt=wpt, b1b=b1b, b2b=b2b, bpb=bpb,
                iota=iota, ident=ident, xt=shared["xt"], degt=shared["degt"])
    in_maps = []
    for c in range(CORES):
        pc = per_core[c]
        m = dict(base)
        m["idx"] = pc["idx"]
        m["rel"] = pc["rel"]
        m["degl"] = pc["degl"]
        m["xtl"] = pc["xtl"]
        in_maps.append(m)
    return in_maps


def _run(inputs, trace=False, tmpdir=None, verbose=True):
    import time
    t0 = time.time()
    def _log(msg):
        if verbose:
            print(f"[kernel {time.time()-t0:7.1f}s] {msg}", flush=True)
    sched, shared, per_core = _host_prep(inputs["x"], inputs["edge_index"])
    _log("host prep done")
    nc = _build_nc(sched)
    _log("build+compile done")
    in_maps = _make_in_maps(sched, shared, per_core,
                            inputs["W1"], inputs["b1"], inputs["W2"],
                            inputs["b2"], inputs["Wp"], inputs["bp"])
    _log("in_maps done")
    core_ids = list(range(CORES))
    if trace:
        # NTFF profiling needs a warm first execute; run once untraced.
        bass_utils.run_bass_kernel_spmd(nc, in_maps, core_ids=core_ids,
                                        trace=False)
        _log("warmup run done")
    res = bass_utils.run_bass_kernel_spmd(nc, in_maps, core_ids=core_ids,
                                          trace=trace, tmpdir=tmpdir)
    _log("run done")
    out = np.empty((N, OUT_C), np.float32)
    for c in range(CORES):
        out[c * S:(c + 1) * S] = res.results[c]["out"][:S]
    return out, res


def kernel(**inputs):
    out, _ = _run(inputs)
    return out


# revision 19
# speedup vs baseline: 1.5622x; 1.2597x over previous
"""GCN encoder (2x GCNConv + linear projection, relu) on 8 Trainium2 cores.

Self-contained: hardcodes the problem shapes (N=50000, E=800000, C=128,
OUT_C=64) and the sharding strategy.  Host side does structural prep only
(edge partitioning/sorting/padding, index-list construction); all FP math
(matmuls, rsqrt, scaling, aggregation, bias, relu) runs on device.

Math identity used on device, per GCNConv layer:
    g = dinv * (x @ W.T)          (dinv = rsqrt(indeg+1), per node)
    out[d] = relu(dinv[d] * (sum_{e: dst=d} g[src_e] + g[d]) + b)
The g[d] self term is computed locally (cheap matmul on the core's own
shard), so the gather list carries only the real edges.

Device mapping per core:
  - nodes sharded by contiguous range (6250/core, padded to 6272)
  - edges partitioned by dst owner, sorted by (dst window of 128, src half)
  - gather: gpsimd dma_gather (bf16 256B rows) from a replicated DRAM table
  - segment-sum: per-128-edge selection-matrix (is_equal vs iota, built 8
    chunks per DVE instruction) matmul accumulating into a
    [128 dst x 128 feat] fp32 PSUM tile/window
  - layer boundary: AllGather of the locally computed scaled table G2
"""

import sys
import numpy as np

for _p in ("/opt/trn_rl_repo",):
    if _p not in sys.path:
        sys.path.append(_p)

import concourse.bacc as bacc
import concourse.tile as tile
from concourse import bass, mybir, bass_utils

F32 = mybir.dt.float32
BF16 = mybir.dt.bfloat16
I16 = mybir.dt.int16
AF = mybir.ActivationFunctionType
ALU = mybir.AluOpType
NP_BF16 = mybir.dt.np(BF16)


class Cfg:
    def __init__(self, n_nodes, n_edges, cores=8, in_c=128, hid_c=128, out_c=64,
                 bf16=True):
        assert in_c == 128 and hid_c == 128
        self.N, self.E, self.CORES = n_nodes, n_edges, cores
        self.C, self.OUT_C = in_c, out_c
        self.BF16 = bf16
        assert n_nodes % cores == 0
        self.S = n_nodes // cores                       # real nodes per shard
        self.SP = -(-self.S // 128) * 128               # padded shard rows
        assert self.SP > self.S, "need pad rows in each shard for zero rows"
        self.NPAD = self.SP * cores                     # padded table rows
        assert self.NPAD % 256 == 0
        self.HALF = self.NPAD // 2                      # int16 table split
        assert self.HALF % self.SP == 0
        assert self.HALF < 32768
        self.NW = self.SP // 128                        # windows per core
        self.NT = self.NPAD // 128                      # node tiles total
        self.GBLK = 16


CFG = Cfg(50000, 800000)


def _wrap16(a):
    """[L] -> [128, L/16] int16 idx layout for dma_gather (16-wrap, 8x repl)."""
    assert a.size % 16 == 0
    w = a.reshape(-1, 16).T.astype(np.int16)
    return np.ascontiguousarray(np.tile(w, (8, 1)))


def _host_prep(cfg, x, edge_index):
    """Build per-core device inputs + the compile-time chunk schedule."""
    N, C = cfg.N, cfg.C
    S, SP, NPAD, HALF, NW, CORES = cfg.S, cfg.SP, cfg.NPAD, cfg.HALF, cfg.NW, cfg.CORES
    tdt = NP_BF16 if cfg.BF16 else np.float32

    src = np.asarray(edge_index[0]).astype(np.int64)
    dst = np.asarray(edge_index[1]).astype(np.int64)
    deg = np.bincount(dst, minlength=N).astype(np.float32) + 1.0

    owner = dst // S
    loc = dst - owner * S
    srcp = (src // S) * SP + (src % S)          # padded global src id
    win = loc // 128
    rel = (loc % 128).astype(np.float32)
    hB = srcp >= HALF

    key = (owner * NW + win) * 2 + hB
    counts = np.bincount(key, minlength=CORES * NW * 2).reshape(CORES, NW, 2)
    maxc = counts.max(axis=0)                           # [NW, 2]
    capA = -(-maxc[:, 0] // 128)                        # S chunks per window
    capB = -(-maxc[:, 1] // 128)
    glenA = -(-maxc[:, 0] // 16) * 16                   # gather idx counts
    glenB = -(-maxc[:, 1] // 16) * 16

    nodes = np.arange(N, dtype=np.int64)
    realpos = (nodes // S) * SP + (nodes % S)
    degp = np.ones(NPAD, np.float32)
    degp[realpos] = deg
    degt = np.ascontiguousarray(degp.reshape(-1, 128).T)          # [128, NT]

    xpad = np.zeros((NPAD, C), np.float32)
    xpad[realpos] = np.asarray(x, np.float32)
    xt = np.ascontiguousarray(xpad.T).astype(tdt)                 # [128, NPAD]

    ZROW = S  # local-to-half id of a guaranteed zero pad row (both halves)

    nchunk = int(capA.sum() + capB.sum())
    nchunk8 = -(-nchunk // 8) * 8

    per_core = []
    for c in range(CORES):
        m = owner == c
        cw, cr, cs, ch = win[m], rel[m], srcp[m], hB[m]
        order = np.lexsort((ch, cw))
        cw, cr, cs, ch = cw[order], cr[order], cs[order], ch[order]
        k = cw * 2 + ch
        ia_parts, ib_parts, rel_parts = [], [], []
        for wi in range(NW):
            for half, cap, glen in ((0, capA[wi], glenA[wi]),
                                    (1, capB[wi], glenB[wi])):
                lo = np.searchsorted(k, wi * 2 + half, "left")
                hi = np.searchsorted(k, wi * 2 + half, "right")
                n = hi - lo
                assert n <= glen <= cap * 128
                iv = cs[lo:hi] - (HALF if half else 0)
                iv = np.concatenate([iv, np.full(glen - n, ZROW, np.int64)])
                rv = np.concatenate(
                    [cr[lo:hi], np.full(cap * 128 - n, -1.0, np.float32)])
                (ib_parts if half else ia_parts).append(iv)
                rel_parts.append(rv)
        rel_parts.append(np.full((nchunk8 - nchunk) * 128, -1.0, np.float32))
        idxa = np.concatenate(ia_parts) if ia_parts else np.zeros(0, np.int64)
        idxb = np.concatenate(ib_parts) if ib_parts else np.zeros(0, np.int64)
        rel_all = np.concatenate(rel_parts).astype(np.float32)
        relT = np.ascontiguousarray(rel_all.reshape(-1, 128).T).astype(tdt)
        degl = np.ascontiguousarray(
            degp[c * SP:(c + 1) * SP].reshape(NW, 128).T)          # [128, NW]
        xtl = np.ascontiguousarray(xt[:, c * SP:(c + 1) * SP])     # [128, SP]
        per_core.append(dict(
            idxa=_wrap16(idxa), idxb=_wrap16(idxb), rel=relT, degl=degl,
            xtl=xtl))

    sched = dict(capA=[int(v) for v in capA], capB=[int(v) for v in capB],
                 glenA=[int(v) for v in glenA], glenB=[int(v) for v in glenB],
                 nchunk8=nchunk8)
    shared = dict(xt=xt, degt=degt)
    return sched, shared, per_core


def _build_nc(cfg, sched):
    C, OUT_C = cfg.C, cfg.OUT_C
    SP, NPAD, HALF, NW, NT, CORES = (cfg.SP, cfg.NPAD, cfg.HALF, cfg.NW,
                                     cfg.NT, cfg.CORES)
    TDT = BF16 if cfg.BF16 else F32
    capA, capB = sched["capA"], sched["capB"]
    glenA, glenB = sched["glenA"], sched["glenB"]
    nchunk8 = sched["nchunk8"]
    la16 = sum(glenA) // 16
    lb16 = sum(glenB) // 16
    gmaxblk = max(
        [min(cfg.GBLK, -(-g // 128)) for g in glenA + glenB if g] or [1])

    nc = bacc.Bacc("TRN2", target_bir_lowering=False, debug=False,
                   enable_asserts=False, num_devices=CORES,
                   num_swdge_queues=4)

    def inp(name, shape, dt=F32):
        return nc.dram_tensor(name, shape, dt, kind="ExternalInput").ap()

    xt_d = inp("xt", [128, NPAD], TDT)
    xtl_d = inp("xtl", [128, SP], TDT)
    w1t_d = inp("w1t", [C, C], TDT)
    w2t_d = inp("w2t", [C, C], TDT)
    wpt_d = inp("wpt", [C, OUT_C], TDT)
    b1b_d = inp("b1b", [128, C])
    b2b_d = inp("b2b", [128, C])
    bpb_d = inp("bpb", [128, OUT_C])
    degt_d = inp("degt", [128, NT])
    degl_d = inp("degl", [128, NW])
    iota_d = inp("iota", [128, 8 * 128], TDT)
    ident_d = inp("ident", [128, 128])
    pmask_d = inp("pmask", [128, 1])
    idxa_d = inp("idxa", [128, max(la16, 16)], I16)
    idxb_d = inp("idxb", [128, max(lb16, 16)], I16)
    rel_d = inp("rel", [128, nchunk8], TDT)
    out_d = nc.dram_tensor("out", [SP, OUT_C], F32, kind="ExternalOutput").ap()

    # layer-1 table split per half so half-A gathers can start while the
    # half-B table is still being built
    g1a = nc.dram_tensor("g1a", [HALF, C], TDT, kind="Internal").ap()
    g1b = nc.dram_tensor("g1b", [HALF, C], TDT, kind="Internal").ap()
    g2loc = nc.dram_tensor("g2loc", [SP, C], TDT, kind="Internal").ap()
    g2d = nc.dram_tensor("g2d", [NPAD, C], TDT, kind="Internal",
                         addr_space="Shared").ap()

    XBLK = 512
    GBLK = cfg.GBLK

    from contextlib import ExitStack
    with tile.TileContext(nc) as tc, ExitStack() as ctx:
        cp = ctx.enter_context(tc.tile_pool(name="consts", bufs=1))
        xpool = ctx.enter_context(tc.tile_pool(name="xload", bufs=3))
        gstp = ctx.enter_context(tc.tile_pool(name="gstage", bufs=3))
        msgp = ctx.enter_context(tc.tile_pool(name="msg", bufs=6))
        spool = ctx.enter_context(tc.tile_pool(name="sel", bufs=6))
        epool = ctx.enter_context(tc.tile_pool(name="epi", bufs=4))
        opool = ctx.enter_context(tc.tile_pool(name="otiles", bufs=1))
        ppool_g = ctx.enter_context(tc.tile_pool(name="psg", bufs=4, space="PSUM"))
        ppool_w = ctx.enter_context(tc.tile_pool(name="psw", bufs=2, space="PSUM"))
        ppool_t = ctx.enter_context(tc.tile_pool(name="pst", bufs=1, space="PSUM"))
        ppool_p = ctx.enter_context(tc.tile_pool(name="psp", bufs=1, space="PSUM"))

        def cload(name, ap, shape, dt=F32):
            t = cp.tile(shape, dt, tag=name)
            nc.sync.dma_start(t[:], ap[:])
            return t

        w1t = cload("w1t", w1t_d, [C, C], TDT)
        w2t = cload("w2t", w2t_d, [C, C], TDT)
        wpt = cload("wpt", wpt_d, [C, OUT_C], TDT)
        b1b = cload("b1b", b1b_d, [128, C])
        b2b = cload("b2b", b2b_d, [128, C])
        bpb = cload("bpb", bpb_d, [128, OUT_C])
        degt = cload("degt", degt_d, [128, NT])
        degl = cload("degl", degl_d, [128, NW])
        iota = cload("iota", iota_d, [128, 8 * 128], TDT)
        ident = cload("ident", ident_d, [128, 128])
        pmask = cload("pmask", pmask_d, [128, 1])
        xtl = cload("xtl", xtl_d, [128, SP], TDT)
        idxa = cload("idxa", idxa_d, [128, max(la16, 16)], I16)
        idxb = cload("idxb", idxb_d, [128, max(lb16, 16)], I16)
        rel = cload("rel", rel_d, [128, nchunk8], TDT)

        # dinv = 1/sqrt(deg) (rsqrt activation is banned for accuracy)
        sqf = cp.tile([128, NT], F32, tag="sqf")
        nc.scalar.activation(sqf[:], degt[:], AF.Sqrt)
        dinv = cp.tile([128, NT], F32, tag="dinv")
        nc.vector.reciprocal(dinv[:], sqf[:])
        sql = cp.tile([128, NW], F32, tag="sql")
        nc.scalar.activation(sql[:], degl[:], AF.Sqrt)
        dinvl = cp.tile([128, NW], F32, tag="dinvl")
        nc.vector.reciprocal(dinvl[:], sql[:])

        # ---- phase G1: full table G1 = dinv * (X @ W1.T), node-major ----
        # half A (tiles 0..NT/2) first, then half B, so A-gathers can start
        for grp in range(NPAD // XBLK):
            xblk = xpool.tile([128, XBLK], TDT, tag="xblk")
            nc.sync.dma_start(xblk[:], xt_d[:, grp * XBLK:(grp + 1) * XBLK])
            gst = gstp.tile([128, XBLK], TDT, tag="gst")
            for j in range(XBLK // 128):
                t = grp * (XBLK // 128) + j
                ps = ppool_g.tile([128, C], F32, tag="psg")
                nc.tensor.matmul(ps[:], lhsT=xblk[:, j * 128:(j + 1) * 128],
                                 rhs=w1t[:], start=True, stop=True)
                # alternate PSUM->SBUF scaled copies between DVE and ACT
                dsl = dinv[:, t:t + 1]
                if t % 2 == 0:
                    nc.vector.tensor_tensor(
                        out=gst[:, j * 128:(j + 1) * 128], in0=ps[:],
                        in1=dsl.to_broadcast([128, 128]), op=ALU.mult)
                else:
                    nc.scalar.activation(
                        gst[:, j * 128:(j + 1) * 128], ps[:], AF.Identity,
                        scale=dsl)
            r0 = grp * XBLK
            tgt = g1a if r0 < HALF else g1b
            r0 = r0 % HALF
            nc.sync.dma_start(
                tgt[r0:r0 + XBLK, :].rearrange("(j p) f -> p j f", p=128),
                gst[:].rearrange("p (j f) -> p j f", f=C))

        # batched selection-matrix construction: 8 chunks per DVE op
        selb_cur = [None]
        cis = [0]

        def sel_for():
            ci = cis[0]
            cis[0] += 1
            if ci % 8 == 0:
                sb = spool.tile([128, 8 * 128], TDT, tag="selb")
                nc.vector.tensor_tensor(
                    out=sb[:].rearrange("p (c f) -> p c f", f=128),
                    in0=rel[:, ci:ci + 8].rearrange(
                        "p (c o) -> p c o", o=1).to_broadcast([128, 8, 128]),
                    in1=iota[:].rearrange("p (c f) -> p c f", f=128),
                    op=ALU.is_equal)
                selb_cur[0] = sb
            k = ci % 8
            return selb_cur[0][:, k * 128:(k + 1) * 128]

        # ---- gather + segment-sum windows (shared for both layers) ----
        def window_phase(tabA, tabB, bias_sb, gl_tiles, otag):
            cis[0] = 0
            selb_cur[0] = None
            # enumerate gather pieces in program order
            pieces = []
            offa = offb = 0   # in idx columns (16 idx each)
            for w in range(NW):
                for half, cap, glen in ((0, capA[w], glenA[w]),
                                        (1, capB[w], glenB[w])):
                    if cap == 0:
                        continue
                    gleft = glen
                    for g0 in range(0, cap, GBLK):
                        gb = min(GBLK, cap - g0)
                        nidx = min(gleft, gb * 128)
                        gleft -= nidx
                        assert nidx > 0
                        off = offa if half == 0 else offb
                        pieces.append((w, half, nidx, off))
                        if half == 0:
                            offa += nidx // 16
                        else:
                            offb += nidx // 16

            def emit_gather(pi):
                w, half, nidx, off = pieces[pi]
                nblk = -(-nidx // 128)
                msg = msgp.tile([128, gmaxblk, C], TDT, tag=f"msg{half}")
                isl = (idxa if half == 0 else idxb)[:, off:off + nidx // 16]
                tab = tabA if half == 0 else tabB
                nc.gpsimd.dma_gather(msg[:, :nblk, :], tab, isl, nidx, nidx,
                                     elem_size=C, single_packet=False,
                                     queue_num=pi % 4)
                return msg

            pi = 0
            otiles = []
            for w in range(NW):
                ps = ppool_w.tile([128, 128], F32, tag="psw")
                nchw = capA[w] + capB[w]
                assert nchw > 0
                done = 0
                for half, cap, glen in ((0, capA[w], glenA[w]),
                                        (1, capB[w], glenB[w])):
                    if cap == 0:
                        continue
                    gleft = glen
                    for g0 in range(0, cap, GBLK):
                        gb = min(GBLK, cap - g0)
                        nidx = min(gleft, gb * 128)
                        gleft -= nidx
                        nblk = -(-nidx // 128)
                        msg = emit_gather(pi)
                        pi += 1
                        for k in range(nblk):
                            # tail chunk: contract only over the partitions
                            # the gather wrote
                            kk = min(128, nidx - k * 128)
                            sel = sel_for()
                            nc.tensor.matmul(ps[:], lhsT=sel[:kk, :],
                                             rhs=msg[:kk, k, :],
                                             start=(done == 0),
                                             stop=(done == nchw - 1))
                            done += 1
                # epilogue: relu(dinv*(acc + g_local) + bias)
                t1 = epool.tile([128, 128], F32, tag="t1")
                nc.vector.tensor_tensor(out=t1[:], in0=ps[:],
                                        in1=gl_tiles[w][:], op=ALU.add)
                t2 = epool.tile([128, 128], F32, tag="t2")
                nc.vector.tensor_tensor(
                    out=t2[:], in0=t1[:],
                    in1=dinvl[:, w:w + 1].to_broadcast([128, 128]),
                    op=ALU.mult)
                t3 = epool.tile([128, 128], F32, tag="t3")
                nc.vector.tensor_tensor(out=t3[:], in0=t2[:], in1=bias_sb[:],
                                        op=ALU.add)
                o = opool.tile([128, 128], F32, tag=f"{otag}_{w}")
                nc.scalar.activation(o[:], t3[:], AF.Relu)
                otiles.append(o)
            return otiles

        # local G1 tiles (self term): dinv_l * (X_local @ W1.T)
        g1l = []
        for w in range(NW):
            ps = ppool_g.tile([128, C], F32, tag="psg")
            nc.tensor.matmul(ps[:], lhsT=xtl[:, w * 128:(w + 1) * 128],
                             rhs=w1t[:], start=True, stop=True)
            gl = opool.tile([128, C], F32, tag=f"gl_{w}")
            nc.scalar.activation(gl[:], ps[:], AF.Identity,
                                 scale=dinvl[:, w:w + 1])
            g1l.append(gl)

        out1 = window_phase(g1a, g1b, b1b, g1l, "o")
        # zero the pad rows of the last tile (they feed G2 through the matmul)
        nc.vector.tensor_tensor(out=out1[NW - 1][:], in0=out1[NW - 1][:],
                                in1=pmask[:, 0:1].to_broadcast([128, 128]),
                                op=ALU.mult)

        # ---- phase G2: local shard table + AllGather ----
        x2t = cp.tile([128, SP], TDT, tag="x2t")
        for w in range(NW):
            pst = ppool_t.tile([128, 128], F32, tag="pst")
            nc.tensor.transpose(pst[:], out1[w][:], ident[:])
            nc.vector.tensor_copy(x2t[:, w * 128:(w + 1) * 128], pst[:])
        g2l = []
        for w in range(NW):
            ps = ppool_g.tile([128, C], F32, tag="psg")
            nc.tensor.matmul(ps[:], lhsT=x2t[:, w * 128:(w + 1) * 128],
                             rhs=w2t[:], start=True, stop=True)
            g2b = epool.tile([128, C], TDT, tag="g2b")
            nc.vector.tensor_tensor(
                out=g2b[:], in0=ps[:],
                in1=dinvl[:, w:w + 1].to_broadcast([128, 128]), op=ALU.mult)
            nc.sync.dma_start(g2loc[w * 128:(w + 1) * 128, :], g2b[:])
            gl = opool.tile([128, C], F32, tag=f"gl_{w}")
            nc.scalar.activation(gl[:], ps[:], AF.Identity,
                                 scale=dinvl[:, w:w + 1])
            g2l.append(gl)
        nc.gpsimd.collective_compute(
            "AllGather", ALU.bypass,
            replica_groups=[list(range(CORES))],
            ins=[g2loc[:]], outs=[g2d[:]])

        out2 = window_phase(g2d[0:HALF, :], g2d[HALF:NPAD, :], b2b, g2l, "o")

        # ---- projection: relu(h2 @ Wp.T + bp) ----
        for w in range(NW):
            pst = ppool_t.tile([128, 128], F32, tag="pst")
            nc.tensor.transpose(pst[:], out2[w][:], ident[:])
            h2t = epool.tile([128, 128], TDT, tag="h2t")
            nc.vector.tensor_copy(h2t[:], pst[:])
            psp = ppool_p.tile([128, OUT_C], F32, tag="psp")
            nc.tensor.matmul(psp[:], lhsT=h2t[:], rhs=wpt[:],
                             start=True, stop=True)
            of = epool.tile([128, OUT_C], F32, tag="of")
            nc.vector.tensor_tensor(out=of[:], in0=psp[:], in1=bpb[:],
                                    op=ALU.add)
            ofr = epool.tile([128, OUT_C], F32, tag="ofr")
            nc.scalar.activation(ofr[:], of[:], AF.Relu)
            nc.sync.dma_start(out_d[w * 128:(w + 1) * 128, :], ofr[:])

    nc.compile()
    return nc


def _make_in_maps(cfg, sched, shared, per_core, W1, b1, W2, b2, Wp, bp):
    tdt = NP_BF16 if cfg.BF16 else np.float32
    w1t = np.ascontiguousarray(np.asarray(W1, np.float32).T).astype(tdt)
    w2t = np.ascontiguousarray(np.asarray(W2, np.float32).T).astype(tdt)
    wpt = np.ascontiguousarray(np.asarray(Wp, np.float32).T).astype(tdt)
    b1b = np.ascontiguousarray(np.tile(np.asarray(b1, np.float32)[None], (128, 1)))
    b2b = np.ascontiguousarray(np.tile(np.asarray(b2, np.float32)[None], (128, 1)))
    bpb = np.ascontiguousarray(np.tile(np.asarray(bp, np.float32)[None], (128, 1)))
    iota = np.tile(np.arange(128, dtype=np.float32)[None], (128, 8)).astype(tdt)
    ident = np.eye(128, dtype=np.float32)
    p0 = cfg.S - (cfg.NW - 1) * 128   # real rows in the last window tile
    pmask = np.ascontiguousarray(
        (np.arange(128) < p0).astype(np.float32)[:, None])
    base = dict(xt=shared["xt"], degt=shared["degt"], w1t=w1t, w2t=w2t,
                wpt=wpt, b1b=b1b, b2b=b2b, bpb=bpb, iota=iota, ident=ident,
                pmask=pmask)
    in_maps = []
    for c in range(cfg.CORES):
        pc = per_core[c]
        m = dict(base)
        m["idxa"] = pc["idxa"] if pc["idxa"].size else np.zeros((128, 16), np.int16)
        m["idxb"] = pc["idxb"] if pc["idxb"].size else np.zeros((128, 16), np.int16)
        m["rel"] = pc["rel"]
        m["degl"] = pc["degl"]
        m["xtl"] = pc["xtl"]
        in_maps.append(m)
    return in_maps


def _run(inputs, cfg=None, trace=False, tmpdir=None, verbose=True):
    import time
    t0 = time.time()
    def _log(msg):
        if verbose:
            print(f"[kernel {time.time()-t0:7.1f}s] {msg}", flush=True)
    cfg = cfg or CFG
    sched, shared, per_core = _host_prep(cfg, inputs["x"], inputs["edge_index"])
    _log("host prep done")
    nc = _build_nc(cfg, sched)
    _log("build+compile done")
    in_maps = _make_in_maps(cfg, sched, shared, per_core,
                            inputs["W1"], inputs["b1"], inputs["W2"],
                            inputs["b2"], inputs["Wp"], inputs["bp"])
    _log("in_maps done")
    core_ids = list(range(cfg.CORES))
    if trace:
        # NTFF profiling needs a warm first execute; run once untraced.
        bass_utils.run_bass_kernel_spmd(nc, in_maps, core_ids=core_ids,
                                        trace=False)
        _log("warmup run done")
    res = bass_utils.run_bass_kernel_spmd(nc, in_maps, core_ids=core_ids,
                                          trace=trace, tmpdir=tmpdir)
    _log("run done")
    out = np.empty((cfg.N, cfg.OUT_C), np.float32)
    for c in range(cfg.CORES):
        out[c * cfg.S:(c + 1) * cfg.S] = res.results[c]["out"][:cfg.S]
    return out, res


def kernel(**inputs):
    out, _ = _run(inputs)
    return out


# revision 20
# speedup vs baseline: 1.5919x; 1.0190x over previous
"""GCN encoder (2x GCNConv + linear projection, relu) on 8 Trainium2 cores.

Self-contained: hardcodes the problem shapes (N=50000, E=800000, C=128,
OUT_C=64) and the sharding strategy.  Host side does structural prep only
(edge partitioning/sorting/padding, index-list construction); all FP math
(matmuls, rsqrt, scaling, aggregation, bias, relu) runs on device.

Math identity used on device, per GCNConv layer:
    g = dinv * (x @ W.T)          (dinv = rsqrt(indeg+1), per node)
    out[d] = relu(dinv[d] * (sum_{e: dst=d} g[src_e] + g[d]) + b)
The g[d] self term is computed locally (cheap matmul on the core's own
shard), so the gather list carries only the real edges.

Device mapping per core:
  - nodes sharded by contiguous range (6250/core, padded to 6272)
  - edges partitioned by dst owner, sorted by (dst window of 128, src half)
  - gather: gpsimd dma_gather (bf16 256B rows) from a replicated DRAM table
  - segment-sum: per-128-edge selection-matrix (is_equal vs iota, built 8
    chunks per DVE instruction) matmul accumulating into a
    [128 dst x 128 feat] fp32 PSUM tile/window
  - layer boundary: AllGather of the locally computed scaled table G2
"""

import sys
import numpy as np

for _p in ("/opt/trn_rl_repo",):
    if _p not in sys.path:
        sys.path.append(_p)

import concourse.bacc as bacc
import concourse.tile as tile
from concourse import bass, mybir, bass_utils

F32 = mybir.dt.float32
BF16 = mybir.dt.bfloat16
I16 = mybir.dt.int16
AF = mybir.ActivationFunctionType
ALU = mybir.AluOpType
NP_BF16 = mybir.dt.np(BF16)


class Cfg:
    def __init__(self, n_nodes, n_edges, cores=8, in_c=128, hid_c=128, out_c=64,
                 bf16=True):
        assert in_c == 128 and hid_c == 128
        self.N, self.E, self.CORES = n_nodes, n_edges, cores
        self.C, self.OUT_C = in_c, out_c
        self.BF16 = bf16
        assert n_nodes % cores == 0
        self.S = n_nodes // cores                       # real nodes per shard
        self.SP = -(-self.S // 128) * 128               # padded shard rows
        assert self.SP > self.S, "need pad rows in each shard for zero rows"
        self.NPAD = self.SP * cores                     # padded table rows
        assert self.NPAD % 256 == 0
        self.HALF = self.NPAD // 2                      # int16 table split
        assert self.HALF % self.SP == 0
        assert self.HALF < 32768
        self.NW = self.SP // 128                        # windows per core
        self.NT = self.NPAD // 128                      # node tiles total
        self.GBLK = 16


CFG = Cfg(50000, 800000)


def _wrap16(a):
    """[L] -> [128, L/16] int16 idx layout for dma_gather (16-wrap, 8x repl)."""
    assert a.size % 16 == 0
    w = a.reshape(-1, 16).T.astype(np.int16)
    return np.ascontiguousarray(np.tile(w, (8, 1)))


def _host_prep(cfg, x, edge_index):
    """Build per-core device inputs + the compile-time chunk schedule."""
    N, C = cfg.N, cfg.C
    S, SP, NPAD, HALF, NW, CORES = cfg.S, cfg.SP, cfg.NPAD, cfg.HALF, cfg.NW, cfg.CORES
    tdt = NP_BF16 if cfg.BF16 else np.float32

    src = np.asarray(edge_index[0]).astype(np.int64)
    dst = np.asarray(edge_index[1]).astype(np.int64)
    deg = np.bincount(dst, minlength=N).astype(np.float32) + 1.0

    owner = dst // S
    loc = dst - owner * S
    srcp = (src // S) * SP + (src % S)          # padded global src id
    win = loc // 128
    rel = (loc % 128).astype(np.float32)
    hB = srcp >= HALF

    key = (owner * NW + win) * 2 + hB
    counts = np.bincount(key, minlength=CORES * NW * 2).reshape(CORES, NW, 2)
    maxc = counts.max(axis=0)                           # [NW, 2]
    capA = -(-maxc[:, 0] // 128)                        # S chunks per window
    capB = -(-maxc[:, 1] // 128)
    glenA = -(-maxc[:, 0] // 16) * 16                   # gather idx counts
    glenB = -(-maxc[:, 1] // 16) * 16

    nodes = np.arange(N, dtype=np.int64)
    realpos = (nodes // S) * SP + (nodes % S)
    degp = np.ones(NPAD, np.float32)
    degp[realpos] = deg
    degt = np.ascontiguousarray(degp.reshape(-1, 128).T)          # [128, NT]

    xpad = np.zeros((NPAD, C), np.float32)
    xpad[realpos] = np.asarray(x, np.float32)
    xt = np.ascontiguousarray(xpad.T).astype(tdt)                 # [128, NPAD]

    ZROW = S  # local-to-half id of a guaranteed zero pad row (both halves)

    nchunk = int(capA.sum() + capB.sum())
    nchunk8 = -(-nchunk // 8) * 8

    per_core = []
    for c in range(CORES):
        m = owner == c
        cw, cr, cs, ch = win[m], rel[m], srcp[m], hB[m]
        order = np.lexsort((ch, cw))
        cw, cr, cs, ch = cw[order], cr[order], cs[order], ch[order]
        k = cw * 2 + ch
        ia_parts, ib_parts, rel_parts = [], [], []
        for wi in range(NW):
            for half, cap, glen in ((0, capA[wi], glenA[wi]),
                                    (1, capB[wi], glenB[wi])):
                lo = np.searchsorted(k, wi * 2 + half, "left")
                hi = np.searchsorted(k, wi * 2 + half, "right")
                n = hi - lo
                assert n <= glen <= cap * 128
                iv = cs[lo:hi] - (HALF if half else 0)
                iv = np.concatenate([iv, np.full(glen - n, ZROW, np.int64)])
                rv = np.concatenate(
                    [cr[lo:hi], np.full(cap * 128 - n, -1.0, np.float32)])
                (ib_parts if half else ia_parts).append(iv)
                rel_parts.append(rv)
        rel_parts.append(np.full((nchunk8 - nchunk) * 128, -1.0, np.float32))
        idxa = np.concatenate(ia_parts) if ia_parts else np.zeros(0, np.int64)
        idxb = np.concatenate(ib_parts) if ib_parts else np.zeros(0, np.int64)
        rel_all = np.concatenate(rel_parts).astype(np.float32)
        relT = np.ascontiguousarray(rel_all.reshape(-1, 128).T).astype(tdt)
        degl = np.ascontiguousarray(
            degp[c * SP:(c + 1) * SP].reshape(NW, 128).T)          # [128, NW]
        xtl = np.ascontiguousarray(xt[:, c * SP:(c + 1) * SP])     # [128, SP]
        per_core.append(dict(
            idxa=_wrap16(idxa), idxb=_wrap16(idxb), rel=relT, degl=degl,
            xtl=xtl))

    sched = dict(capA=[int(v) for v in capA], capB=[int(v) for v in capB],
                 glenA=[int(v) for v in glenA], glenB=[int(v) for v in glenB],
                 nchunk8=nchunk8)
    shared = dict(xt=xt, degt=degt)
    return sched, shared, per_core


def _build_nc(cfg, sched):
    C, OUT_C = cfg.C, cfg.OUT_C
    SP, NPAD, HALF, NW, NT, CORES = (cfg.SP, cfg.NPAD, cfg.HALF, cfg.NW,
                                     cfg.NT, cfg.CORES)
    TDT = BF16 if cfg.BF16 else F32
    capA, capB = sched["capA"], sched["capB"]
    glenA, glenB = sched["glenA"], sched["glenB"]
    nchunk8 = sched["nchunk8"]
    la16 = sum(glenA) // 16
    lb16 = sum(glenB) // 16
    gmaxblk = max(
        [min(cfg.GBLK, -(-g // 128)) for g in glenA + glenB if g] or [1])

    nc = bacc.Bacc("TRN2", target_bir_lowering=False, debug=False,
                   enable_asserts=False, num_devices=CORES,
                   num_swdge_queues=4)

    def inp(name, shape, dt=F32):
        return nc.dram_tensor(name, shape, dt, kind="ExternalInput").ap()

    xt_d = inp("xt", [128, NPAD], TDT)
    xtl_d = inp("xtl", [128, SP], TDT)
    w1t_d = inp("w1t", [C, C], TDT)
    w2t_d = inp("w2t", [C, C], TDT)
    wpt_d = inp("wpt", [C, OUT_C], TDT)
    b1b_d = inp("b1b", [128, C])
    b2b_d = inp("b2b", [128, C])
    bpb_d = inp("bpb", [128, OUT_C])
    degt_d = inp("degt", [128, NT])
    degl_d = inp("degl", [128, NW])
    iota_d = inp("iota", [128, 8 * 128], TDT)
    ident_d = inp("ident", [128, 128])
    pmask_d = inp("pmask", [128, 1])
    idxa_d = inp("idxa", [128, max(la16, 16)], I16)
    idxb_d = inp("idxb", [128, max(lb16, 16)], I16)
    rel_d = inp("rel", [128, nchunk8], TDT)
    out_d = nc.dram_tensor("out", [SP, OUT_C], F32, kind="ExternalOutput").ap()

    # layer-1 table split per half so half-A gathers can start while the
    # half-B table is still being built
    g1a = nc.dram_tensor("g1a", [HALF, C], TDT, kind="Internal").ap()
    g1b = nc.dram_tensor("g1b", [HALF, C], TDT, kind="Internal").ap()
    g2loc = nc.dram_tensor("g2loc", [SP, C], TDT, kind="Internal").ap()
    g2d = nc.dram_tensor("g2d", [NPAD, C], TDT, kind="Internal",
                         addr_space="Shared").ap()

    XBLK = 512
    GBLK = cfg.GBLK

    from contextlib import ExitStack
    with tile.TileContext(nc) as tc, ExitStack() as ctx:
        cp = ctx.enter_context(tc.tile_pool(name="consts", bufs=1))
        xpool = ctx.enter_context(tc.tile_pool(name="xload", bufs=3))
        gstp = ctx.enter_context(tc.tile_pool(name="gstage", bufs=3))
        msgp = ctx.enter_context(tc.tile_pool(name="msg", bufs=8))
        spool = ctx.enter_context(tc.tile_pool(name="sel", bufs=6))
        epool = ctx.enter_context(tc.tile_pool(name="epi", bufs=4))
        opool = ctx.enter_context(tc.tile_pool(name="otiles", bufs=1))
        ppool_g = ctx.enter_context(tc.tile_pool(name="psg", bufs=4, space="PSUM"))
        ppool_w = ctx.enter_context(tc.tile_pool(name="psw", bufs=2, space="PSUM"))
        ppool_t = ctx.enter_context(tc.tile_pool(name="pst", bufs=1, space="PSUM"))
        ppool_p = ctx.enter_context(tc.tile_pool(name="psp", bufs=1, space="PSUM"))

        def cload(name, ap, shape, dt=F32):
            t = cp.tile(shape, dt, tag=name)
            nc.sync.dma_start(t[:], ap[:])
            return t

        w1t = cload("w1t", w1t_d, [C, C], TDT)
        w2t = cload("w2t", w2t_d, [C, C], TDT)
        wpt = cload("wpt", wpt_d, [C, OUT_C], TDT)
        b1b = cload("b1b", b1b_d, [128, C])
        b2b = cload("b2b", b2b_d, [128, C])
        bpb = cload("bpb", bpb_d, [128, OUT_C])
        degt = cload("degt", degt_d, [128, NT])
        degl = cload("degl", degl_d, [128, NW])
        iota = cload("iota", iota_d, [128, 8 * 128], TDT)
        ident = cload("ident", ident_d, [128, 128])
        pmask = cload("pmask", pmask_d, [128, 1])
        xtl = cload("xtl", xtl_d, [128, SP], TDT)
        idxa = cload("idxa", idxa_d, [128, max(la16, 16)], I16)
        idxb = cload("idxb", idxb_d, [128, max(lb16, 16)], I16)
        rel = cload("rel", rel_d, [128, nchunk8], TDT)

        # dinv = 1/sqrt(deg) (rsqrt activation is banned for accuracy)
        sqf = cp.tile([128, NT], F32, tag="sqf")
        nc.scalar.activation(sqf[:], degt[:], AF.Sqrt)
        dinv = cp.tile([128, NT], F32, tag="dinv")
        nc.vector.reciprocal(dinv[:], sqf[:])
        sql = cp.tile([128, NW], F32, tag="sql")
        nc.scalar.activation(sql[:], degl[:], AF.Sqrt)
        dinvl = cp.tile([128, NW], F32, tag="dinvl")
        nc.vector.reciprocal(dinvl[:], sql[:])

        # ---- phase G1: full table G1 = dinv * (X @ W1.T), node-major ----
        # half A (tiles 0..NT/2) first, then half B, so A-gathers can start
        for grp in range(NPAD // XBLK):
            xblk = xpool.tile([128, XBLK], TDT, tag="xblk")
            nc.sync.dma_start(xblk[:], xt_d[:, grp * XBLK:(grp + 1) * XBLK])
            gst = gstp.tile([128, XBLK], TDT, tag="gst")
            for j in range(XBLK // 128):
                t = grp * (XBLK // 128) + j
                ps = ppool_g.tile([128, C], F32, tag="psg")
                nc.tensor.matmul(ps[:], lhsT=xblk[:, j * 128:(j + 1) * 128],
                                 rhs=w1t[:], start=True, stop=True)
                # alternate PSUM->SBUF scaled copies between DVE and ACT
                dsl = dinv[:, t:t + 1]
                if t % 2 == 0:
                    nc.vector.tensor_tensor(
                        out=gst[:, j * 128:(j + 1) * 128], in0=ps[:],
                        in1=dsl.to_broadcast([128, 128]), op=ALU.mult)
                else:
                    nc.scalar.activation(
                        gst[:, j * 128:(j + 1) * 128], ps[:], AF.Identity,
                        scale=dsl)
            r0 = grp * XBLK
            tgt = g1a if r0 < HALF else g1b
            r0 = r0 % HALF
            nc.sync.dma_start(
                tgt[r0:r0 + XBLK, :].rearrange("(j p) f -> p j f", p=128),
                gst[:].rearrange("p (j f) -> p j f", f=C))

        # batched selection-matrix construction: 8 chunks per DVE op
        selb_cur = [None]
        cis = [0]

        def sel_for():
            ci = cis[0]
            cis[0] += 1
            if ci % 8 == 0:
                sb = spool.tile([128, 8 * 128], TDT, tag="selb")
                nc.vector.tensor_tensor(
                    out=sb[:].rearrange("p (c f) -> p c f", f=128),
                    in0=rel[:, ci:ci + 8].rearrange(
                        "p (c o) -> p c o", o=1).to_broadcast([128, 8, 128]),
                    in1=iota[:].rearrange("p (c f) -> p c f", f=128),
                    op=ALU.is_equal)
                selb_cur[0] = sb
            k = ci % 8
            return selb_cur[0][:, k * 128:(k + 1) * 128]

        # ---- gather + segment-sum windows (shared for both layers) ----
        def window_phase(tabA, tabB, bias_sb, gl_tiles, otag):
            cis[0] = 0
            selb_cur[0] = None
            # enumerate gather pieces in program order
            pieces = []
            offa = offb = 0   # in idx columns (16 idx each)
            for w in range(NW):
                for half, cap, glen in ((0, capA[w], glenA[w]),
                                        (1, capB[w], glenB[w])):
                    if cap == 0:
                        continue
                    gleft = glen
                    for g0 in range(0, cap, GBLK):
                        gb = min(GBLK, cap - g0)
                        nidx = min(gleft, gb * 128)
                        gleft -= nidx
                        assert nidx > 0
                        off = offa if half == 0 else offb
                        pieces.append((w, half, nidx, off))
                        if half == 0:
                            offa += nidx // 16
                        else:
                            offb += nidx // 16

            def emit_gather(pi):
                w, half, nidx, off = pieces[pi]
                nblk = -(-nidx // 128)
                msg = msgp.tile([128, gmaxblk, C], TDT, tag=f"msg{half}")
                isl = (idxa if half == 0 else idxb)[:, off:off + nidx // 16]
                tab = tabA if half == 0 else tabB
                nc.gpsimd.dma_gather(msg[:, :nblk, :], tab, isl, nidx, nidx,
                                     elem_size=C, single_packet=False,
                                     queue_num=pi % 4)
                return msg

            pi = 0
            otiles = []
            for w in range(NW):
                ps = ppool_w.tile([128, 128], F32, tag="psw")
                nchw = capA[w] + capB[w]
                assert nchw > 0
                done = 0
                for half, cap, glen in ((0, capA[w], glenA[w]),
                                        (1, capB[w], glenB[w])):
                    if cap == 0:
                        continue
                    gleft = glen
                    for g0 in range(0, cap, GBLK):
                        gb = min(GBLK, cap - g0)
                        nidx = min(gleft, gb * 128)
                        gleft -= nidx
                        nblk = -(-nidx // 128)
                        msg = emit_gather(pi)
                        pi += 1
                        for k in range(nblk):
                            # tail chunk: contract only over the partitions
                            # the gather wrote
                            kk = min(128, nidx - k * 128)
                            sel = sel_for()
                            nc.tensor.matmul(ps[:], lhsT=sel[:kk, :],
                                             rhs=msg[:kk, k, :],
                                             start=(done == 0),
                                             stop=(done == nchw - 1))
                            done += 1
                # epilogue: relu(dinv*(acc + g_local) + bias)
                t1 = epool.tile([128, 128], F32, tag="t1")
                nc.vector.tensor_tensor(out=t1[:], in0=ps[:],
                                        in1=gl_tiles[w][:], op=ALU.add)
                t2 = epool.tile([128, 128], F32, tag="t2")
                nc.vector.tensor_tensor(
                    out=t2[:], in0=t1[:],
                    in1=dinvl[:, w:w + 1].to_broadcast([128, 128]),
                    op=ALU.mult)
                t3 = epool.tile([128, 128], F32, tag="t3")
                nc.vector.tensor_tensor(out=t3[:], in0=t2[:], in1=bias_sb[:],
                                        op=ALU.add)
                o = opool.tile([128, 128], F32, tag=f"{otag}_{w}")
                nc.scalar.activation(o[:], t3[:], AF.Relu)
                otiles.append(o)
            return otiles

        # local G1 tiles (self term): dinv_l * (X_local @ W1.T)
        g1l = []
        for w in range(NW):
            ps = ppool_g.tile([128, C], F32, tag="psg")
            nc.tensor.matmul(ps[:], lhsT=xtl[:, w * 128:(w + 1) * 128],
                             rhs=w1t[:], start=True, stop=True)
            gl = opool.tile([128, C], F32, tag=f"gl_{w}")
            nc.scalar.activation(gl[:], ps[:], AF.Identity,
                                 scale=dinvl[:, w:w + 1])
            g1l.append(gl)

        out1 = window_phase(g1a, g1b, b1b, g1l, "o")
        # zero the pad rows of the last tile (they feed G2 through the matmul)
        nc.vector.tensor_tensor(out=out1[NW - 1][:], in0=out1[NW - 1][:],
                                in1=pmask[:, 0:1].to_broadcast([128, 128]),
                                op=ALU.mult)

        # ---- phase G2: local shard table + AllGather ----
        x2t = cp.tile([128, SP], TDT, tag="x2t")
        for w in range(NW):
            pst = ppool_t.tile([128, 128], F32, tag="pst")
            nc.tensor.transpose(pst[:], out1[w][:], ident[:])
            nc.vector.tensor_copy(x2t[:, w * 128:(w + 1) * 128], pst[:])
        g2l = []
        for w in range(NW):
            ps = ppool_g.tile([128, C], F32, tag="psg")
            nc.tensor.matmul(ps[:], lhsT=x2t[:, w * 128:(w + 1) * 128],
                             rhs=w2t[:], start=True, stop=True)
            g2b = epool.tile([128, C], TDT, tag="g2b")
            nc.vector.tensor_tensor(
                out=g2b[:], in0=ps[:],
                in1=dinvl[:, w:w + 1].to_broadcast([128, 128]), op=ALU.mult)
            nc.sync.dma_start(g2loc[w * 128:(w + 1) * 128, :], g2b[:])
            gl = opool.tile([128, C], F32, tag=f"gl_{w}")
            nc.scalar.activation(gl[:], ps[:], AF.Identity,
                                 scale=dinvl[:, w:w + 1])
            g2l.append(gl)
        nc.gpsimd.collective_compute(
            "AllGather", ALU.bypass,
            replica_groups=[list(range(CORES))],
            ins=[g2loc[:]], outs=[g2d[:]])

        out2 = window_phase(g2d[0:HALF, :], g2d[HALF:NPAD, :], b2b, g2l, "o")

        # ---- projection: relu(h2 @ Wp.T + bp) ----
        for w in range(NW):
            pst = ppool_t.tile([128, 128], F32, tag="pst")
            nc.tensor.transpose(pst[:], out2[w][:], ident[:])
            h2t = epool.tile([128, 128], TDT, tag="h2t")
            nc.vector.tensor_copy(h2t[:], pst[:])
            psp = ppool_p.tile([128, OUT_C], F32, tag="psp")
            nc.tensor.matmul(psp[:], lhsT=h2t[:], rhs=wpt[:],
                             start=True, stop=True)
            of = epool.tile([128, OUT_C], F32, tag="of")
            nc.vector.tensor_tensor(out=of[:], in0=psp[:], in1=bpb[:],
                                    op=ALU.add)
            ofr = epool.tile([128, OUT_C], F32, tag="ofr")
            nc.scalar.activation(ofr[:], of[:], AF.Relu)
            nc.sync.dma_start(out_d[w * 128:(w + 1) * 128, :], ofr[:])

    nc.compile()
    return nc


def _make_in_maps(cfg, sched, shared, per_core, W1, b1, W2, b2, Wp, bp):
    tdt = NP_BF16 if cfg.BF16 else np.float32
    w1t = np.ascontiguousarray(np.asarray(W1, np.float32).T).astype(tdt)
    w2t = np.ascontiguousarray(np.asarray(W2, np.float32).T).astype(tdt)
    wpt = np.ascontiguousarray(np.asarray(Wp, np.float32).T).astype(tdt)
    b1b = np.ascontiguousarray(np.tile(np.asarray(b1, np.float32)[None], (128, 1)))
    b2b = np.ascontiguousarray(np.tile(np.asarray(b2, np.float32)[None], (128, 1)))
    bpb = np.ascontiguousarray(np.tile(np.asarray(bp, np.float32)[None], (128, 1)))
    iota = np.tile(np.arange(128, dtype=np.float32)[None], (128, 8)).astype(tdt)
    ident = np.eye(128, dtype=np.float32)
    p0 = cfg.S - (cfg.NW - 1) * 128   # real rows in the last window tile
    pmask = np.ascontiguousarray(
        (np.arange(128) < p0).astype(np.float32)[:, None])
    base = dict(xt=shared["xt"], degt=shared["degt"], w1t=w1t, w2t=w2t,
                wpt=wpt, b1b=b1b, b2b=b2b, bpb=bpb, iota=iota, ident=ident,
                pmask=pmask)
    in_maps = []
    for c in range(cfg.CORES):
        pc = per_core[c]
        m = dict(base)
        m["idxa"] = pc["idxa"] if pc["idxa"].size else np.zeros((128, 16), np.int16)
        m["idxb"] = pc["idxb"] if pc["idxb"].size else np.zeros((128, 16), np.int16)
        m["rel"] = pc["rel"]
        m["degl"] = pc["degl"]
        m["xtl"] = pc["xtl"]
        in_maps.append(m)
    return in_maps


def _run(inputs, cfg=None, trace=False, tmpdir=None, verbose=True):
    import time
    t0 = time.time()
    def _log(msg):
        if verbose:
            print(f"[kernel {time.time()-t0:7.1f}s] {msg}", flush=True)
    cfg = cfg or CFG
    sched, shared, per_core = _host_prep(cfg, inputs["x"], inputs["edge_index"])
    _log("host prep done")
    nc = _build_nc(cfg, sched)
    _log("build+compile done")
    in_maps = _make_in_maps(cfg, sched, shared, per_core,
                            inputs["W1"], inputs["b1"], inputs["W2"],
                            inputs["b2"], inputs["Wp"], inputs["bp"])
    _log("in_maps done")
    core_ids = list(range(cfg.CORES))
    if trace:
        # NTFF profiling needs a warm first execute; run once untraced.
        bass_utils.run_bass_kernel_spmd(nc, in_maps, core_ids=core_ids,
                                        trace=False)
        _log("warmup run done")
    res = bass_utils.run_bass_kernel_spmd(nc, in_maps, core_ids=core_ids,
                                          trace=trace, tmpdir=tmpdir)
    _log("run done")
    out = np.empty((cfg.N, cfg.OUT_C), np.float32)
    for c in range(cfg.CORES):
        out[c * cfg.S:(c + 1) * cfg.S] = res.results[c]["out"][:cfg.S]
    return out, res


def kernel(**inputs):
    out, _ = _run(inputs)
    return out
